# revision 10
# baseline (speedup 1.0000x reference)
"""Contextual-attention Trainium2 kernel (Bass/Tile), data-parallel over batch.

Math (per sequence b):
    Q = evo @ q_w.T + q_b                                  (L, 96)
    K = cat(evo, conv3(evo), conv5(evo)) @ k_w.T + k_b     (L, 96)
    V = plm @ v_w.T + v_b                                  (L, 96)
    P = softmax(Q K^T / sqrt(96), key-masked by seqlen)
    out = P @ V + V

Device-side reformulation (per core = one sequence):
  * Convs + concat + K-projection fold into 5 shifted matmuls (host-folded
    taps):  K[l] = sum_{t=-2..2} evo[l+t] @ taps[t] + kb
  * Everything transposed ([feature, L] layout):
        QT = wq.T @ evoT, KT = taps.T @ evoT(shifted), VT = wv.T @ plmT
        ST[lk, lq] = KT_tile.T @ QT  -> ET = exp(NORM*ST + maskbias) in fp8
        OT[0:96], OT[96] = sum_lk V1[lk].T @ ET[lk]  (V1 = [V | ones] fp8)
  * fp8e4m3 + MatmulPerfMode.DoubleRow doubles effective PE throughput for
    contraction-bound matmuls (2 x 128-partition k-subtiles per instruction):
    used for the Q/K projections (evo + weights fp8) and the P@V accumulation
    (ET + V1 fp8).  The scores matmul contracts over only 96 dims and is
    column-stream-bound either way, so QT/KT stay fp16 for accuracy.  The V
    projection stays fp16 because V feeds the residual directly.
  * Q/K/V biases are applied during the PSUM->SBUF drains (per-partition
    scalar add on DVE); the key mask is the exp bias (0 / -1e6) so masked
    keys underflow to exactly 0.0, matching the reference's where()+softmax.
  * DMA trigger instructions cost ~600ns each on the issuing engine's queue,
    so inputs travel in a handful of large descriptors spread across the
    sync/vector/gpsimd queues.  Dummy matmuls at t=0 spin the PE p-state up
    while the first DMAs land.
  * Scalar-engine exp (~1.1us per [128,1024] tile, 28 tiles) is the #2
    critical path after PE; emission order keeps it fed from ~4.5us: all
    query-half-0 exps first (so O half-0 can run early), V/transposes/O-h0
    interleaved into the S-half-1 stretch where PE idles on exp backpressure.
  * Final divide by denominator, +V residual, and the (96, L) -> (L, 96)
    transpose happen on host (tiny O(L*96) work).
"""

import os
import numpy as np
import ml_dtypes

import concourse.bacc as bacc
import concourse.bass as bass
import concourse.tile as tile
from concourse import mybir
from concourse._compat import get_trn_type
from concourse.bass_utils import run_bass_kernel_spmd

B, L = 8, 2048
Q_IN, V_IN, QK, VD = 512, 1024, 96, 96
P = 128
HALF = L // 2
EC = 1028
WQO = 1056
WKO = 1152
BQO = 1632
EW = 1664
PW = 2272
NORM = float(1.0 / np.sqrt(QK))
F32 = mybir.dt.float32
F16 = mybir.dt.float16
F8 = mybir.dt.float8e4
NPF8 = ml_dtypes.float8_e4m3
DR = mybir.MatmulPerfMode.DoubleRow

LAST_EXEC_TIME_NS = None
LAST_RESULTS = None

_program_cache = {}


def _fold_k_weights(k_w, k_b, cn3_w, cn3_b, cn5_w, cn5_b):
    """K[l] = sum_{t in -2..2} evo[l+t] @ taps[t+2] + bk  (zero-padded shifts)."""
    A_evo = k_w[:, :Q_IN]
    A3 = k_w[:, Q_IN : Q_IN + VD]
    A5 = k_w[:, Q_IN + VD :]
    taps = np.zeros((5, Q_IN, QK), np.float32)
    for j in range(3):  # conv3 tap j acts at offset t = j-1
        taps[j - 1 + 2] += np.einsum("oc,cd->do", A3, cn3_w[:, :, j]).astype(np.float32)
    for j in range(5):  # conv5 tap j acts at offset t = j-2
        taps[j - 2 + 2] += np.einsum("oc,cd->do", A5, cn5_w[:, :, j]).astype(np.float32)
    taps[2] += A_evo.T
    bk = (k_b + A3 @ cn3_b + A5 @ cn5_b).astype(np.float32)
    return taps, bk


def _chunks(total, step=512):
    out = []
    o = 0
    while o < total:
        out.append((o, min(step, total - o)))
        o += step
    return out


def _build_program(nkt):
    """One SPMD program; all cores run NKT key tiles, masks differ per core."""
    lkw = nkt * P
    nc = bacc.Bacc(get_trn_type() or "TRN2", target_bir_lowering=False, debug=False)
    # params: two packed blobs -- every DMA packet is one partition segment
    # (~500ns each regardless of size), so small tensors ride inside the blobs.
    # evoAug per (p,c): [evoA 0:1028 | wq_c 1028:1124 | wk_c,t 1124:1604 | pad]
    # with bqkv as f32 bitcast at cols 1604:1616 of c=0.
    # plmAug per (p,g): [plm 0:2048 | wv_g 2048:2144 | extras 2144:2272]
    # with mask (f32) in g=4 extras.
    evoAd = nc.declare_dram_parameter("evoA", [P, 4, EW], F8, isOutput=False)
    evoBd = nc.declare_dram_parameter("evoB", [P, 4, EC], F8, isOutput=False)
    plmd = nc.declare_dram_parameter("plm", [P, 8, PW], F16, isOutput=False)
    # outputs (both fp16; host divides / transposes / adds residual)
    ot_out = nc.declare_dram_parameter("ot", [QK + 1, L], F16, isOutput=True)
    vt_out = nc.declare_dram_parameter("vt", [QK, L], F16, isOutput=True)

    add = mybir.AluOpType.add

    with tile.TileContext(nc) as tc:
        with tc.tile_pool(name="sing", bufs=1) as sing:
            # ---- resident tiles ----
            dummy = sing.tile([P, 512], F16, tag="dummy")
            evoA_sb = sing.tile([P, 4, EW], F8, tag="evoA")
            evoB_sb = sing.tile([P, 4, EC], F8, tag="evoB")
            plm_sb = sing.tile([P, 8, PW], F16, tag="plm")
            v116_sb = sing.tile([P, nkt, QK], F16, tag="v116")
            b_sb = evoA_sb[:QK, 0, BQO : BQO + 12].bitcast(F32)
            mask_sb = plm_sb[:, 4, 2144 : 2144 + 2 * nkt].bitcast(F32)
            qt_sb = sing.tile([QK, L], F16, tag="qt")
            kt_sb = sing.tile([QK, lkw], F16, tag="kt")
            vt_sb = sing.tile([QK, L], F16, tag="vt")
            v1_sb = sing.tile([P, nkt, P], F8, tag="v1")
            et_sb = sing.tile([P, nkt, L], F8, tag="et")
            ot_sb = sing.tile([QK + 1, L], F16, tag="ot")

            # ---- t=0: warm-up + DMA triggers spread across engine queues ----
            nc.gpsimd.memset(dummy, 0.0)
            nc.gpsimd.memset(v1_sb[:, :, QK : QK + 1], 1.0)
            nc.gpsimd.memset(v1_sb[:, :, QK + 1 :], 0.0)
            # partition-group triggers across the 3 DGE queues; each blob
            # descriptor is per-partition contiguous
            nc.sync.dma_start(out=evoA_sb[:64], in_=evoAd[:64, :, :])
            nc.scalar.dma_start(out=evoA_sb[64:], in_=evoAd[64:, :, :])
            nc.gpsimd.dma_start(out=plm_sb[:, 4:, :], in_=plmd[:, 4:, :])
            nc.sync.dma_start(out=evoB_sb, in_=evoBd[:, :, :])
            nc.scalar.dma_start(out=plm_sb[:, :4, :], in_=plmd[:, :4, :])
            # preload the Act exp table during the DMA wait
            nc.scalar.activation(
                out=dummy.bitcast(F32)[:1, :1],
                in_=dummy.bitcast(F32)[:1, :1],
                func=mybir.ActivationFunctionType.Exp,
                bias=0.0,
                scale=1.0,
            )

            with tc.tile_pool(name="warm_ps", bufs=1, space="PSUM") as warm_ps:
                wt = warm_ps.tile([P, 512], F32, tag="warm")
                for _ in range(7):
                    nc.tensor.matmul(
                        wt, lhsT=dummy[:, :P], rhs=dummy, start=True, stop=True
                    )

            with (
                tc.tile_pool(name="proj_ps", bufs=2, space="PSUM") as proj_ps,
                tc.tile_pool(name="st_ps", bufs=2, space="PSUM") as st_ps,
            ):
                # ---------- emission helpers ----------
                def q_chunk(base, width):
                    eb, off = (evoA_sb, 0) if base < 1024 else (evoB_sb, 1024)
                    pt = proj_ps.tile([P, 1024], F32, tag="b")
                    for c in (0, 2):
                        nc.tensor.matmul(
                            pt[:QK, :width],
                            lhsT=evoA_sb[:, c : c + 2, WQO : WQO + QK],
                            rhs=eb[:, c : c + 2, 2 + base - off : 2 + base - off + width],
                            start=(c == 0),
                            stop=(c == 2),
                            perf_mode=DR,
                        )
                    nc.vector.tensor_scalar(
                        out=qt_sb[:, base : base + width],
                        in0=pt[:QK, :width],
                        scalar1=b_sb[:, 0:1],
                        scalar2=None,
                        op0=add,
                    )

                def k_chunk(base, width):
                    eb, off = (evoA_sb, 0) if base < 1024 else (evoB_sb, 1024)
                    pt = proj_ps.tile([P, 1024], F32, tag="b")
                    n = 0
                    for t in range(5):
                        for c in (0, 2):
                            nc.tensor.matmul(
                                pt[:QK, :width],
                                lhsT=evoA_sb[
                                    :, c : c + 2, WKO + QK * t : WKO + QK * (t + 1)
                                ],
                                rhs=eb[:, c : c + 2, t + base - off : t + base - off + width],
                                start=(n == 0),
                                stop=(n == 9),
                                perf_mode=DR,
                            )
                            n += 1
                    nc.vector.tensor_scalar(
                        out=kt_sb[:, base : base + width],
                        in0=pt[:QK, :width],
                        scalar1=b_sb[:, 1:2],
                        scalar2=None,
                        op0=add,
                    )

                def s_tile(j, h):
                    hb = h * HALF
                    stp = st_ps.tile([P, HALF], F32, tag="stp")
                    for o2, w2 in _chunks(HALF, 512):
                        nc.tensor.matmul(
                            stp[:, o2 : o2 + w2],
                            lhsT=kt_sb[:, j * P : (j + 1) * P],
                            rhs=qt_sb[:, hb + o2 : hb + o2 + w2],
                            start=True,
                            stop=True,
                        )
                    nc.scalar.activation(
                        out=et_sb[:, j, hb : hb + HALF],
                        in_=stp,
                        func=mybir.ActivationFunctionType.Exp,
                        bias=mask_sb[:, j : j + 1],
                        scale=NORM,
                    )

                def v_chunk(base, width):
                    pt = proj_ps.tile([P, 1024], F32, tag="b")
                    for c in range(8):
                        nc.tensor.matmul(
                            pt[:QK, :width],
                            lhsT=plm_sb[:, c, 2048 : 2048 + QK],
                            rhs=plm_sb[:, c, base : base + width],
                            start=(c == 0),
                            stop=(c == 7),
                        )
                    nc.vector.tensor_scalar(
                        out=vt_sb[:, base : base + width],
                        in0=pt[:QK, :width],
                        scalar1=b_sb[:, 2:3],
                        scalar2=None,
                        op0=add,
                    )

                def v1_tile(j):
                    # XBAR dma transpose (no PE time), then fp8 cast on DVE
                    nc.sync.dma_start_transpose(
                        v116_sb[:, j, :], vt_sb[:, j * P : (j + 1) * P]
                    )
                    nc.vector.tensor_copy(
                        out=v1_sb[:, j, :QK], in_=v116_sb[:, j, :]
                    )

                pairs = [(j, min(2, nkt - j)) for j in range(0, nkt, 2)]

                def o_half(h):
                    hb = h * HALF
                    otp = proj_ps.tile([P, 1024], F32, tag="b")
                    for o2, w2 in _chunks(HALF, 512):
                        for pi, (j, m) in enumerate(pairs):
                            if m == 2:
                                nc.tensor.matmul(
                                    otp[:, o2 : o2 + w2],
                                    lhsT=v1_sb[:, j : j + 2, :],
                                    rhs=et_sb[:, j : j + 2, hb + o2 : hb + o2 + w2],
                                    start=(pi == 0),
                                    stop=(pi == len(pairs) - 1),
                                    perf_mode=DR,
                                )
                            else:
                                nc.tensor.matmul(
                                    otp[:, o2 : o2 + w2],
                                    lhsT=v1_sb[:, j, :],
                                    rhs=et_sb[:, j, hb + o2 : hb + o2 + w2],
                                    start=(pi == 0),
                                    stop=(pi == len(pairs) - 1),
                                )
                    # split drain across DVE + Act (both near-idle at the tail)
                    nc.vector.tensor_copy(
                        out=ot_sb[:, hb : hb + 512], in_=otp[: QK + 1, :512]
                    )
                    nc.scalar.copy(
                        out=ot_sb[:, hb + 512 : hb + HALF],
                        in_=otp[: QK + 1, 512:HALF],
                    )

                # ---------- emission ----------
                # S(j, h0) needs only QT cols 0..1023 (chunks 0/1, from evoA)
                q_chunk(0, 512)
                q_chunk(512, 512)

                # K chunks interleaved with S(h0) for newly covered key tiles;
                # Q chunks 2/3 (evoB) slot in after the first S batch
                done_j = 0
                kch = _chunks(lkw, 512)
                for ci, (base, width) in enumerate(kch):
                    k_chunk(base, width)
                    avail = (base + width) // P
                    for j in range(done_j, avail):
                        s_tile(j, 0)
                    done_j = avail
                    if ci == 0:
                        q_chunk(1024, 512)
                        q_chunk(1536, 512)

                # S(h1) with V proj / V1 transposes / O(h0) interleaved into
                # the exp-backpressure stalls
                jobs2 = (
                    [("v", c) for c in _chunks(L, 512)]
                    + [("x", j) for j in range(nkt)]
                    + [("vtout", 0), ("vtout", 1)]
                    + [("o0", None)]
                )
                sh1 = list(range(nkt))
                ji = 0
                # lead with a couple of S(h1) tiles so the Act queue never dries
                for j in sh1[:2]:
                    s_tile(j, 1)
                for idx, j in enumerate(sh1[2:], start=2):
                    s_tile(j, 1)
                    # ~1 background job per S tile keeps PE busy during waits
                    take = 2 if idx >= len(sh1) - 3 else 1
                    for _ in range(take):
                        if ji < len(jobs2):
                            kind, arg = jobs2[ji]
                            ji += 1
                            if kind == "v":
                                v_chunk(*arg)
                            elif kind == "x":
                                v1_tile(arg)
                            elif kind == "vtout":
                                h = arg
                                nc.gpsimd.dma_start(
                                    out=vt_out[:, h * HALF : (h + 1) * HALF],
                                    in_=vt_sb[:, h * HALF : (h + 1) * HALF],
                                )
                            else:
                                o_half(0)
                while ji < len(jobs2):
                    kind, arg = jobs2[ji]
                    ji += 1
                    if kind == "v":
                        v_chunk(*arg)
                    elif kind == "x":
                        v1_tile(arg)
                    elif kind == "vtout":
                        h = arg
                        nc.gpsimd.dma_start(
                            out=vt_out[:, h * HALF : (h + 1) * HALF],
                            in_=vt_sb[:, h * HALF : (h + 1) * HALF],
                        )
                    else:
                        o_half(0)
                # O(h0) outputs
                nc.gpsimd.dma_start(out=ot_out[:48, :HALF], in_=ot_sb[:48, :HALF])
                nc.sync.dma_start(
                    out=ot_out[48 : QK + 1, :HALF], in_=ot_sb[48 : QK + 1, :HALF]
                )
                # O(h1): final output split across all three queues
                o_half(1)
                nc.gpsimd.dma_start(out=ot_out[:32, HALF:], in_=ot_sb[:32, HALF:])
                nc.sync.dma_start(out=ot_out[32:64, HALF:], in_=ot_sb[32:64, HALF:])
                nc.scalar.dma_start(
                    out=ot_out[64 : QK + 1, HALF:], in_=ot_sb[64 : QK + 1, HALF:]
                )
    nc.finalize()
    return nc


def _prep_core_inputs(evo, plm, seqlen, weights, nkt):
    evoT = np.zeros((Q_IN, L + 4), np.float32)
    evoT[:, 2 : 2 + L] = evo.T
    evo8 = evoT.reshape(4, P, L + 4).transpose(1, 0, 2).astype(NPF8)
    evoA = np.zeros((P, 4, EW), NPF8)
    evoA[:, :, :EC] = evo8[:, :, :EC]
    evoA[:, :, WQO : WQO + QK] = weights["wq"]
    evoA[:, :, WKO : WKO + 5 * QK] = weights["wk"]
    evoA[:QK, 0, BQO : BQO + 12].view(np.float32)[:] = weights["bqkv"]
    evoB = np.ascontiguousarray(evo8[:, :, L + 4 - EC :])
    plm16 = np.zeros((P, 8, PW), np.float16)
    plm16[:, :, :L] = plm.T.reshape(8, P, L).transpose(1, 0, 2)
    plm16[:, :, 2048 : 2048 + QK] = weights["wv"]
    j = np.arange(nkt)[None, :]
    p = np.arange(P)[:, None]
    mask = np.where(j * P + p < seqlen, 0.0, -1e6).astype(np.float32)
    plm16[:, 4, 2144 : 2144 + 2 * nkt].view(np.float32)[:] = mask
    return {"evoA": evoA, "evoB": evoB, "plm": plm16}


def _pack_w(w, n, dt):
    # (n*128, 96) f32 -> (128, n, 96) in the SBUF [p, n, o] layout
    return np.ascontiguousarray(w.reshape(n, P, QK).transpose(1, 0, 2)).astype(dt)


def kernel(
    plm_embedding,
    evo_local,
    seqlengths,
    q_w,
    q_b,
    k_w,
    k_b,
    v_w,
    v_b,
    cn3_w,
    cn3_b,
    cn5_w,
    cn5_b,
):
    global LAST_EXEC_TIME_NS, LAST_RESULTS
    plm_embedding = np.asarray(plm_embedding, np.float32)
    evo_local = np.asarray(evo_local, np.float32)
    seqlengths = np.asarray(seqlengths)

    taps, bk = _fold_k_weights(
        np.asarray(k_w, np.float32),
        np.asarray(k_b, np.float32),
        np.asarray(cn3_w, np.float32),
        np.asarray(cn3_b, np.float32),
        np.asarray(cn5_w, np.float32),
        np.asarray(cn5_b, np.float32),
    )
    nkt = int(min(L // P, (int(seqlengths.max()) + P - 1) // P))
    bqkv = np.stack(
        [np.asarray(q_b, np.float32), bk, np.asarray(v_b, np.float32)], axis=1
    ).astype(np.float32)
    wk20 = _pack_w(taps.reshape(5 * Q_IN, QK), 20, NPF8)  # [P, 20, 96] t-major
    weights = {
        "wq": _pack_w(np.ascontiguousarray(np.asarray(q_w, np.float32).T), 4, NPF8),
        # blob wants [P, c, t, 96]: c-major pairs with taps adjacent per c
        "wk": np.ascontiguousarray(
            wk20.reshape(P, 5, 4, QK).transpose(0, 2, 1, 3).reshape(P, 4, 5 * QK)
        ),
        "wv": _pack_w(
            np.ascontiguousarray(np.asarray(v_w, np.float32).T), 8, np.float16
        ),
        "bqkv": np.ascontiguousarray(bqkv),
    }

    if nkt not in _program_cache:
        _program_cache[nkt] = _build_program(nkt)
    nc = _program_cache[nkt]

    in_maps = [
        _prep_core_inputs(
            evo_local[b], plm_embedding[b], int(seqlengths[b]), weights, nkt
        )
        for b in range(B)
    ]
    trace = bool(os.environ.get("KBENCH_TRACE"))
    res = run_bass_kernel_spmd(nc, in_maps, list(range(B)), trace=trace)
    LAST_EXEC_TIME_NS = res.exec_time_ns
    LAST_RESULTS = res

    out = np.empty((B, L, VD), np.float32)
    for b in range(B):
        ot = res.results[b]["ot"].astype(np.float32)
        vt = res.results[b]["vt"].astype(np.float32)
        out[b] = (ot[:QK] / ot[QK : QK + 1]).T + vt.T
    return out


# revision 11
# speedup vs baseline: 1.2874x; 1.2874x over previous
"""Contextual-attention Trainium2 kernel (Bass/Tile), data-parallel over batch.

Math (per sequence b):
    Q = evo @ q_w.T + q_b                                  (L, 96)
    K = cat(evo, conv3(evo), conv5(evo)) @ k_w.T + k_b     (L, 96)
    V = plm @ v_w.T + v_b                                  (L, 96)
    P = softmax(Q K^T / sqrt(96), key-masked by seqlen)
    out = P @ V + V

Device-side reformulation (per core = one sequence):
  * Convs + concat + K-projection fold into 5 shifted matmuls (host-folded
    taps):  K[l] = sum_{t=-2..2} evo[l+t] @ taps[t] + kb
  * Everything transposed ([feature, L] layout):
        QT = wq.T @ evoT, KT = taps.T @ evoT(shifted), VT = wv.T @ plmT
        ST[lk, lq] = KT_tile.T @ QT  -> ET = exp(NORM*ST + maskbias) in fp8
        OT[0:96], OT[96] = sum_lk V1[lk].T @ ET[lk]  (V1 = [V | ones] fp8)
  * fp8e4m3 + MatmulPerfMode.DoubleRow doubles effective PE throughput for
    contraction-bound matmuls (2 x 128-partition k-subtiles per instruction):
    used for the Q/K projections (evo + weights fp8) and the P@V accumulation
    (ET + V1 fp8).  The scores matmul contracts over only 96 dims and is
    column-stream-bound either way, so QT/KT stay fp16 for accuracy.  The V
    projection stays fp16 because V feeds the residual directly.
  * Q/K/V biases are applied during the PSUM->SBUF drains (per-partition
    scalar add on DVE); the key mask is the exp bias (0 / -1e6) so masked
    keys underflow to exactly 0.0, matching the reference's where()+softmax.
  * DMA trigger instructions cost ~600ns each on the issuing engine's queue,
    so inputs travel in a handful of large descriptors spread across the
    sync/vector/gpsimd queues.  Dummy matmuls at t=0 spin the PE p-state up
    while the first DMAs land.
  * Scalar-engine exp (~1.1us per [128,1024] tile, 28 tiles) is the #2
    critical path after PE; emission order keeps it fed from ~4.5us: all
    query-half-0 exps first (so O half-0 can run early), V/transposes/O-h0
    interleaved into the S-half-1 stretch where PE idles on exp backpressure.
  * Final divide by denominator, +V residual, and the (96, L) -> (L, 96)
    transpose happen on host (tiny O(L*96) work).
"""

import os
import numpy as np
import ml_dtypes

import concourse.bacc as bacc
import concourse.bass as bass
import concourse.tile as tile
from concourse import mybir
from concourse._compat import get_trn_type
from concourse.bass_utils import run_bass_kernel_spmd

B, L = 8, 2048
Q_IN, V_IN, QK, VD = 512, 1024, 96, 96
P = 128
HALF = L // 2
EC = 1028
WQO = 1056
WKO = 1152
BQO = 1632
EW = 1664
PW = 2272
NORM = float(1.0 / np.sqrt(QK))
F32 = mybir.dt.float32
F16 = mybir.dt.float16
F8 = mybir.dt.float8e4
NPF8 = ml_dtypes.float8_e4m3
DR = mybir.MatmulPerfMode.DoubleRow

LAST_EXEC_TIME_NS = None
LAST_RESULTS = None

_program_cache = {}


def _fold_k_weights(k_w, k_b, cn3_w, cn3_b, cn5_w, cn5_b):
    """K[l] = sum_{t in -2..2} evo[l+t] @ taps[t+2] + bk  (zero-padded shifts)."""
    A_evo = k_w[:, :Q_IN]
    A3 = k_w[:, Q_IN : Q_IN + VD]
    A5 = k_w[:, Q_IN + VD :]
    taps = np.zeros((5, Q_IN, QK), np.float32)
    for j in range(3):  # conv3 tap j acts at offset t = j-1
        taps[j - 1 + 2] += np.einsum("oc,cd->do", A3, cn3_w[:, :, j]).astype(np.float32)
    for j in range(5):  # conv5 tap j acts at offset t = j-2
        taps[j - 2 + 2] += np.einsum("oc,cd->do", A5, cn5_w[:, :, j]).astype(np.float32)
    taps[2] += A_evo.T
    bk = (k_b + A3 @ cn3_b + A5 @ cn5_b).astype(np.float32)
    return taps, bk


def _chunks(total, step=512):
    out = []
    o = 0
    while o < total:
        out.append((o, min(step, total - o)))
        o += step
    return out


def _build_program(nkt):
    """One SPMD program; all cores run NKT key tiles, masks differ per core."""
    lkw = nkt * P
    nc = bacc.Bacc(get_trn_type() or "TRN2", target_bir_lowering=False, debug=False)
    # params: two packed blobs -- every DMA packet is one partition segment
    # (~500ns each regardless of size), so small tensors ride inside the blobs.
    # evoAug per (p,c): [evoA 0:1028 | wq_c 1028:1124 | wk_c,t 1124:1604 | pad]
    # with bqkv as f32 bitcast at cols 1604:1616 of c=0.
    # plmAug per (p,g): [plm 0:2048 | wv_g 2048:2144 | extras 2144:2272]
    # with mask (f32) in g=4 extras.
    evoAd = nc.declare_dram_parameter("evoA", [P, 4, EW], F8, isOutput=False)
    evoBd = nc.declare_dram_parameter("evoB", [P, 4, EC], F8, isOutput=False)
    plmd = nc.declare_dram_parameter("plm", [P, 8, PW], F16, isOutput=False)
    # outputs (both fp16; host divides / transposes / adds residual)
    ot_out = nc.declare_dram_parameter("ot", [QK + 1, L], F16, isOutput=True)
    vt_out = nc.declare_dram_parameter("vt", [QK, L], F16, isOutput=True)

    add = mybir.AluOpType.add

    with tile.TileContext(nc) as tc:
        with tc.tile_pool(name="sing", bufs=1) as sing:
            # ---- resident tiles ----
            dummy = sing.tile([P, 512], F16, tag="dummy")
            evoA_sb = sing.tile([P, 4, EW], F8, tag="evoA")
            evoB_sb = sing.tile([P, 4, EC], F8, tag="evoB")
            plm_sb = sing.tile([P, 8, PW], F16, tag="plm")
            b_sb = evoA_sb[:QK, 0, BQO : BQO + 12].bitcast(F32)
            mask_sb = plm_sb[:, 4, 2144 : 2144 + 2 * nkt].bitcast(F32)
            qt_sb = sing.tile([QK, L], F16, tag="qt")
            kt_sb = sing.tile([QK, lkw], F16, tag="kt")
            vt_sb = sing.tile([QK, L], F16, tag="vt")
            v1_sb = sing.tile([P, nkt, P], F8, tag="v1")
            et_sb = sing.tile([P, nkt, L], F8, tag="et")
            ot_sb = sing.tile([QK + 1, L], F16, tag="ot")

            # ---- t=0: warm-up + DMA triggers spread across engine queues ----
            nc.gpsimd.memset(dummy, 0.0)
            nc.gpsimd.memset(v1_sb[:, :, QK : QK + 1], 1.0)
            nc.gpsimd.memset(v1_sb[:, :, QK + 1 :], 0.0)
            # partition-group triggers across the 3 DGE queues; each blob
            # descriptor is per-partition contiguous
            nc.sync.dma_start(out=evoA_sb[:64], in_=evoAd[:64, :, :])
            nc.scalar.dma_start(out=evoA_sb[64:], in_=evoAd[64:, :, :])
            nc.gpsimd.dma_start(out=plm_sb[:, 4:, :], in_=plmd[:, 4:, :])
            nc.sync.dma_start(out=evoB_sb, in_=evoBd[:, :, :])
            nc.scalar.dma_start(out=plm_sb[:, :4, :], in_=plmd[:, :4, :])
            # preload the Act exp table during the DMA wait
            nc.scalar.activation(
                out=dummy.bitcast(F32)[:1, :1],
                in_=dummy.bitcast(F32)[:1, :1],
                func=mybir.ActivationFunctionType.Exp,
                bias=0.0,
                scale=1.0,
            )

            with tc.tile_pool(name="warm_ps", bufs=1, space="PSUM") as warm_ps:
                wt = warm_ps.tile([P, 512], F32, tag="warm")
                for _ in range(7):
                    nc.tensor.matmul(
                        wt, lhsT=dummy[:, :P], rhs=dummy, start=True, stop=True
                    )

            with (
                tc.tile_pool(name="proj_ps", bufs=2, space="PSUM") as proj_ps,
                tc.tile_pool(name="st_ps", bufs=2, space="PSUM") as st_ps,
            ):
                # ---------- emission helpers ----------
                def q_chunk(base, width):
                    eb, off = (evoA_sb, 0) if base < 1024 else (evoB_sb, 1024)
                    pt = proj_ps.tile([P, 1024], F32, tag="b")
                    for c in (0, 2):
                        nc.tensor.matmul(
                            pt[:QK, :width],
                            lhsT=evoA_sb[:, c : c + 2, WQO : WQO + QK],
                            rhs=eb[:, c : c + 2, 2 + base - off : 2 + base - off + width],
                            start=(c == 0),
                            stop=(c == 2),
                            perf_mode=DR,
                        )
                    nc.vector.tensor_scalar(
                        out=qt_sb[:, base : base + width],
                        in0=pt[:QK, :width],
                        scalar1=b_sb[:, 0:1],
                        scalar2=None,
                        op0=add,
                    )

                def k_chunk(base, width):
                    eb, off = (evoA_sb, 0) if base < 1024 else (evoB_sb, 1024)
                    pt = proj_ps.tile([P, 1024], F32, tag="b")
                    n = 0
                    for t in range(5):
                        for c in (0, 2):
                            nc.tensor.matmul(
                                pt[:QK, :width],
                                lhsT=evoA_sb[
                                    :, c : c + 2, WKO + QK * t : WKO + QK * (t + 1)
                                ],
                                rhs=eb[:, c : c + 2, t + base - off : t + base - off + width],
                                start=(n == 0),
                                stop=(n == 9),
                                perf_mode=DR,
                            )
                            n += 1
                    nc.vector.tensor_scalar(
                        out=kt_sb[:, base : base + width],
                        in0=pt[:QK, :width],
                        scalar1=b_sb[:, 1:2],
                        scalar2=None,
                        op0=add,
                    )

                def s_tile(j, h):
                    hb = h * HALF
                    stp = st_ps.tile([P, HALF], F32, tag="stp")
                    for o2, w2 in _chunks(HALF, 512):
                        nc.tensor.matmul(
                            stp[:, o2 : o2 + w2],
                            lhsT=kt_sb[:, j * P : (j + 1) * P],
                            rhs=qt_sb[:, hb + o2 : hb + o2 + w2],
                            start=True,
                            stop=True,
                        )
                    nc.scalar.activation(
                        out=et_sb[:, j, hb : hb + HALF],
                        in_=stp,
                        func=mybir.ActivationFunctionType.Exp,
                        bias=mask_sb[:, j : j + 1],
                        scale=NORM,
                    )

                def v_chunk(base, width):
                    pt = proj_ps.tile([P, 1024], F32, tag="b")
                    for c in range(8):
                        nc.tensor.matmul(
                            pt[:QK, :width],
                            lhsT=plm_sb[:, c, 2048 : 2048 + QK],
                            rhs=plm_sb[:, c, base : base + width],
                            start=(c == 0),
                            stop=(c == 7),
                        )
                    nc.vector.tensor_scalar(
                        out=vt_sb[:, base : base + width],
                        in0=pt[:QK, :width],
                        scalar1=b_sb[:, 2:3],
                        scalar2=None,
                        op0=add,
                    )

                def v1_tile(j):
                    # PE transpose into a shared-pool slot viewed as fp16
                    ptf = proj_ps.tile([P, 1024], F32, tag="b")
                    vp = ptf.bitcast(F16)[:, :QK]
                    nc.tensor.transpose(
                        vp,
                        vt_sb[:, j * P : (j + 1) * P],
                        plm_sb[:QK, 1, 2144 : 2144 + QK],
                    )
                    nc.vector.tensor_copy(out=v1_sb[:, j, :QK], in_=vp)

                pairs = [(j, min(2, nkt - j)) for j in range(0, nkt, 2)]

                def o_half(h):
                    hb = h * HALF
                    otp = proj_ps.tile([P, 1024], F32, tag="b")
                    for o2, w2 in _chunks(HALF, 512):
                        for pi, (j, m) in enumerate(pairs):
                            if m == 2:
                                nc.tensor.matmul(
                                    otp[:, o2 : o2 + w2],
                                    lhsT=v1_sb[:, j : j + 2, :],
                                    rhs=et_sb[:, j : j + 2, hb + o2 : hb + o2 + w2],
                                    start=(pi == 0),
                                    stop=(pi == len(pairs) - 1),
                                    perf_mode=DR,
                                )
                            else:
                                nc.tensor.matmul(
                                    otp[:, o2 : o2 + w2],
                                    lhsT=v1_sb[:, j, :],
                                    rhs=et_sb[:, j, hb + o2 : hb + o2 + w2],
                                    start=(pi == 0),
                                    stop=(pi == len(pairs) - 1),
                                )
                    # split drain across DVE + Act (both near-idle at the tail)
                    nc.vector.tensor_copy(
                        out=ot_sb[:, hb : hb + 512], in_=otp[: QK + 1, :512]
                    )
                    nc.scalar.copy(
                        out=ot_sb[:, hb + 512 : hb + HALF],
                        in_=otp[: QK + 1, 512:HALF],
                    )

                # ---------- emission ----------
                # S(j, h0) needs only QT cols 0..1023 (chunks 0/1, from evoA)
                q_chunk(0, 512)
                q_chunk(512, 512)

                # K chunks interleaved with S(h0) for newly covered key tiles;
                # Q chunks 2/3 (evoB) slot in after the first S batch
                done_j = 0
                kch = _chunks(lkw, 512)
                for ci, (base, width) in enumerate(kch):
                    k_chunk(base, width)
                    avail = (base + width) // P
                    for j in range(done_j, avail):
                        s_tile(j, 0)
                    done_j = avail
                    if ci == 0:
                        q_chunk(1024, 512)
                        q_chunk(1536, 512)

                # S(h1) with V proj / V1 transposes / O(h0) interleaved into
                # the exp-backpressure stalls
                jobs2 = (
                    [("v", c) for c in _chunks(L, 512)]
                    + [("x", j) for j in range(nkt)]
                    + [("vtout", 0), ("vtout", 1)]
                    + [("o0", None)]
                )
                sh1 = list(range(nkt))
                ji = 0
                # lead with a couple of S(h1) tiles so the Act queue never dries
                for j in sh1[:2]:
                    s_tile(j, 1)
                for idx, j in enumerate(sh1[2:], start=2):
                    s_tile(j, 1)
                    # ~1 background job per S tile keeps PE busy during waits
                    take = 2 if idx >= len(sh1) - 3 else 1
                    for _ in range(take):
                        if ji < len(jobs2):
                            kind, arg = jobs2[ji]
                            ji += 1
                            if kind == "v":
                                v_chunk(*arg)
                            elif kind == "x":
                                v1_tile(arg)
                            elif kind == "vtout":
                                h = arg
                                nc.gpsimd.dma_start(
                                    out=vt_out[:, h * HALF : (h + 1) * HALF],
                                    in_=vt_sb[:, h * HALF : (h + 1) * HALF],
                                )
                            else:
                                o_half(0)
                while ji < len(jobs2):
                    kind, arg = jobs2[ji]
                    ji += 1
                    if kind == "v":
                        v_chunk(*arg)
                    elif kind == "x":
                        v1_tile(arg)
                    elif kind == "vtout":
                        h = arg
                        nc.gpsimd.dma_start(
                            out=vt_out[:, h * HALF : (h + 1) * HALF],
                            in_=vt_sb[:, h * HALF : (h + 1) * HALF],
                        )
                    else:
                        o_half(0)
                # O(h0) outputs
                nc.gpsimd.dma_start(out=ot_out[:48, :HALF], in_=ot_sb[:48, :HALF])
                nc.sync.dma_start(
                    out=ot_out[48 : QK + 1, :HALF], in_=ot_sb[48 : QK + 1, :HALF]
                )
                # O(h1): final output split across all three queues
                o_half(1)
                nc.gpsimd.dma_start(out=ot_out[:32, HALF:], in_=ot_sb[:32, HALF:])
                nc.sync.dma_start(out=ot_out[32:64, HALF:], in_=ot_sb[32:64, HALF:])
                nc.scalar.dma_start(
                    out=ot_out[64 : QK + 1, HALF:], in_=ot_sb[64 : QK + 1, HALF:]
                )
    nc.finalize()
    return nc


def _prep_core_inputs(evo, plm, seqlen, weights, nkt):
    evoT = np.zeros((Q_IN, L + 4), np.float32)
    evoT[:, 2 : 2 + L] = evo.T
    evo8 = evoT.reshape(4, P, L + 4).transpose(1, 0, 2).astype(NPF8)
    evoA = np.zeros((P, 4, EW), NPF8)
    evoA[:, :, :EC] = evo8[:, :, :EC]
    evoA[:, :, WQO : WQO + QK] = weights["wq"]
    evoA[:, :, WKO : WKO + 5 * QK] = weights["wk"]
    evoA[:QK, 0, BQO : BQO + 12].view(np.float32)[:] = weights["bqkv"]
    evoB = np.ascontiguousarray(evo8[:, :, L + 4 - EC :])
    plm16 = np.zeros((P, 8, PW), np.float16)
    plm16[:, :, :L] = plm.T.reshape(8, P, L).transpose(1, 0, 2)
    plm16[:, :, 2048 : 2048 + QK] = weights["wv"]
    j = np.arange(nkt)[None, :]
    p = np.arange(P)[:, None]
    mask = np.where(j * P + p < seqlen, 0.0, -1e6).astype(np.float32)
    plm16[:, 4, 2144 : 2144 + 2 * nkt].view(np.float32)[:] = mask
    plm16[:QK, 1, 2144 : 2144 + QK] = np.eye(QK, dtype=np.float16)
    return {"evoA": evoA, "evoB": evoB, "plm": plm16}


def _pack_w(w, n, dt):
    # (n*128, 96) f32 -> (128, n, 96) in the SBUF [p, n, o] layout
    return np.ascontiguousarray(w.reshape(n, P, QK).transpose(1, 0, 2)).astype(dt)


def kernel(
    plm_embedding,
    evo_local,
    seqlengths,
    q_w,
    q_b,
    k_w,
    k_b,
    v_w,
    v_b,
    cn3_w,
    cn3_b,
    cn5_w,
    cn5_b,
):
    global LAST_EXEC_TIME_NS, LAST_RESULTS
    plm_embedding = np.asarray(plm_embedding, np.float32)
    evo_local = np.asarray(evo_local, np.float32)
    seqlengths = np.asarray(seqlengths)

    taps, bk = _fold_k_weights(
        np.asarray(k_w, np.float32),
        np.asarray(k_b, np.float32),
        np.asarray(cn3_w, np.float32),
        np.asarray(cn3_b, np.float32),
        np.asarray(cn5_w, np.float32),
        np.asarray(cn5_b, np.float32),
    )
    nkt = int(min(L // P, (int(seqlengths.max()) + P - 1) // P))
    bqkv = np.stack(
        [np.asarray(q_b, np.float32), bk, np.asarray(v_b, np.float32)], axis=1
    ).astype(np.float32)
    wk20 = _pack_w(taps.reshape(5 * Q_IN, QK), 20, NPF8)  # [P, 20, 96] t-major
    weights = {
        "wq": _pack_w(np.ascontiguousarray(np.asarray(q_w, np.float32).T), 4, NPF8),
        # blob wants [P, c, t, 96]: c-major pairs with taps adjacent per c
        "wk": np.ascontiguousarray(
            wk20.reshape(P, 5, 4, QK).transpose(0, 2, 1, 3).reshape(P, 4, 5 * QK)
        ),
        "wv": _pack_w(
            np.ascontiguousarray(np.asarray(v_w, np.float32).T), 8, np.float16
        ),
        "bqkv": np.ascontiguousarray(bqkv),
    }

    if nkt not in _program_cache:
        _program_cache[nkt] = _build_program(nkt)
    nc = _program_cache[nkt]

    in_maps = [
        _prep_core_inputs(
            evo_local[b], plm_embedding[b], int(seqlengths[b]), weights, nkt
        )
        for b in range(B)
    ]
    trace = bool(os.environ.get("KBENCH_TRACE"))
    res = run_bass_kernel_spmd(nc, in_maps, list(range(B)), trace=trace)
    LAST_EXEC_TIME_NS = res.exec_time_ns
    LAST_RESULTS = res

    out = np.empty((B, L, VD), np.float32)
    for b in range(B):
        ot = res.results[b]["ot"].astype(np.float32)
        vt = res.results[b]["vt"].astype(np.float32)
        out[b] = (ot[:QK] / ot[QK : QK + 1]).T + vt.T
    return out


# revision 12
# speedup vs baseline: 1.4979x; 1.1635x over previous
"""Contextual-attention Trainium2 kernel (Bass/Tile), data-parallel over batch.

Math (per sequence b):
    Q = evo @ q_w.T + q_b                                  (L, 96)
    K = cat(evo, conv3(evo), conv5(evo)) @ k_w.T + k_b     (L, 96)
    V = plm @ v_w.T + v_b                                  (L, 96)
    P = softmax(Q K^T / sqrt(96), key-masked by seqlen)
    out = P @ V + V

Device-side reformulation (per core = one sequence):
  * Convs + concat + K-projection fold into 5 shifted matmuls (host-folded
    taps):  K[l] = sum_{t=-2..2} evo[l+t] @ taps[t] + kb
  * Everything transposed ([feature, L] layout):
        QT = wq.T @ evoT, KT = taps.T @ evoT(shifted), VT = wv.T @ plmT
        ST[lk, lq] = KT_tile.T @ QT  -> ET = exp(NORM*ST + maskbias) in fp8
        OT[0:96], OT[96] = sum_lk V1[lk].T @ ET[lk]  (V1 = [V | ones] fp8)
  * fp8e4m3 + MatmulPerfMode.DoubleRow doubles effective PE throughput for
    contraction-bound matmuls (2 x 128-partition k-subtiles per instruction):
    used for the Q/K projections (evo + weights fp8) and the P@V accumulation
    (ET + V1 fp8).  The scores matmul contracts over only 96 dims and is
    column-stream-bound either way, so QT/KT stay fp16 for accuracy.  The V
    projection stays fp16 because V feeds the residual directly.
  * Q/K/V biases are applied during the PSUM->SBUF drains (per-partition
    scalar add on DVE); the key mask is the exp bias (0 / -1e6) so masked
    keys underflow to exactly 0.0, matching the reference's where()+softmax.
  * DMA trigger instructions cost ~600ns each on the issuing engine's queue,
    so inputs travel in a handful of large descriptors spread across the
    sync/vector/gpsimd queues.  Dummy matmuls at t=0 spin the PE p-state up
    while the first DMAs land.
  * Scalar-engine exp (~1.1us per [128,1024] tile, 28 tiles) is the #2
    critical path after PE; emission order keeps it fed from ~4.5us: all
    query-half-0 exps first (so O half-0 can run early), V/transposes/O-h0
    interleaved into the S-half-1 stretch where PE idles on exp backpressure.
  * Final divide by denominator, +V residual, and the (96, L) -> (L, 96)
    transpose happen on host (tiny O(L*96) work).
"""

import os
import numpy as np
import ml_dtypes

import concourse.bacc as bacc
import concourse.bass as bass
import concourse.tile as tile
from concourse import mybir
from concourse._compat import get_trn_type
from concourse.bass_utils import run_bass_kernel_spmd

B, L = 8, 2048
Q_IN, V_IN, QK, VD = 512, 1024, 96, 96
P = 128
HALF = L // 2
EC = 1028
WQO = 1056
WKO = 1152
BQO = 1632
MKO = 1648
EW = 1728
PW = 2272
NORM = float(1.0 / np.sqrt(QK))
F32 = mybir.dt.float32
F16 = mybir.dt.float16
F8 = mybir.dt.float8e4
NPF8 = ml_dtypes.float8_e4m3
DR = mybir.MatmulPerfMode.DoubleRow

LAST_EXEC_TIME_NS = None
LAST_RESULTS = None

_program_cache = {}


def _fold_k_weights(k_w, k_b, cn3_w, cn3_b, cn5_w, cn5_b):
    """K[l] = sum_{t in -2..2} evo[l+t] @ taps[t+2] + bk  (zero-padded shifts)."""
    A_evo = k_w[:, :Q_IN]
    A3 = k_w[:, Q_IN : Q_IN + VD]
    A5 = k_w[:, Q_IN + VD :]
    taps = np.zeros((5, Q_IN, QK), np.float32)
    for j in range(3):  # conv3 tap j acts at offset t = j-1
        taps[j - 1 + 2] += np.einsum("oc,cd->do", A3, cn3_w[:, :, j]).astype(np.float32)
    for j in range(5):  # conv5 tap j acts at offset t = j-2
        taps[j - 2 + 2] += np.einsum("oc,cd->do", A5, cn5_w[:, :, j]).astype(np.float32)
    taps[2] += A_evo.T
    bk = (k_b + A3 @ cn3_b + A5 @ cn5_b).astype(np.float32)
    return taps, bk


def _chunks(total, step=512):
    out = []
    o = 0
    while o < total:
        out.append((o, min(step, total - o)))
        o += step
    return out


def _build_program(nkt):
    """One SPMD program; all cores run NKT key tiles, masks differ per core."""
    lkw = nkt * P
    nc = bacc.Bacc(get_trn_type() or "TRN2", target_bir_lowering=False, debug=False)
    # params: two packed blobs -- every DMA packet is one partition segment
    # (~500ns each regardless of size), so small tensors ride inside the blobs.
    # evoAug per (p,c): [evoA 0:1028 | wq_c 1028:1124 | wk_c,t 1124:1604 | pad]
    # with bqkv as f32 bitcast at cols 1604:1616 of c=0.
    # plmAug per (p,g): [plm 0:2048 | wv_g 2048:2144 | extras 2144:2272]
    # with mask (f32) in g=4 extras.
    evoAd = nc.declare_dram_parameter("evoA", [P, 4, EW], F8, isOutput=False)
    evoBd = nc.declare_dram_parameter("evoB", [P, 4, EC], F8, isOutput=False)
    plmd = nc.declare_dram_parameter("plm", [P, 8, PW], F16, isOutput=False)
    # outputs (both fp16; host divides / transposes / adds residual)
    ot_out = nc.declare_dram_parameter("ot", [QK + 1, L], F16, isOutput=True)
    vt_out = nc.declare_dram_parameter("vt", [QK, L], F16, isOutput=True)

    add = mybir.AluOpType.add

    with tile.TileContext(nc) as tc:
        with tc.tile_pool(name="sing", bufs=1) as sing:
            # ---- resident tiles ----
            dummy = sing.tile([P, 512], F16, tag="dummy")
            evoA_sb = sing.tile([P, 4, EW], F8, tag="evoA")
            evoB_sb = sing.tile([P, 4, EC], F8, tag="evoB")
            plm_sb = sing.tile([P, 8, PW], F16, tag="plm")
            b_sb = evoA_sb[:QK, 0, BQO : BQO + 12].bitcast(F32)
            mask_sb = evoA_sb[:, 0, MKO : MKO + 4 * nkt].bitcast(F32)
            qt_sb = sing.tile([QK, L], F16, tag="qt")
            kt_sb = sing.tile([QK, lkw], F16, tag="kt")
            vt_sb = sing.tile([QK, L], F16, tag="vt")
            v1_sb = sing.tile([P, nkt, P], F8, tag="v1")
            et_sb = sing.tile([P, nkt, L], F8, tag="et")
            ot_sb = sing.tile([QK + 1, L], F16, tag="ot")

            # ---- t=0: warm-up + DMA triggers spread across engine queues ----
            nc.gpsimd.memset(dummy, 0.0)
            nc.gpsimd.memset(v1_sb[:, :, QK : QK + 1], 1.0)
            nc.gpsimd.memset(v1_sb[:, :, QK + 1 :], 0.0)
            # partition-group triggers across the 3 DGE queues; each blob
            # descriptor is per-partition contiguous
            nc.sync.dma_start(out=evoA_sb[:64], in_=evoAd[:64, :, :])
            nc.scalar.dma_start(out=evoA_sb[64:], in_=evoAd[64:, :, :])
            nc.gpsimd.dma_start(out=evoB_sb, in_=evoBd[:, :, :])
            nc.scalar.dma_start(out=plm_sb[:, :4, :], in_=plmd[:, :4, :])
            nc.gpsimd.dma_start(out=plm_sb[:, 4:, :], in_=plmd[:, 4:, :])
            # preload the Act exp table during the DMA wait
            nc.scalar.activation(
                out=dummy.bitcast(F32)[:1, :1],
                in_=dummy.bitcast(F32)[:1, :1],
                func=mybir.ActivationFunctionType.Exp,
                bias=0.0,
                scale=1.0,
            )

            with tc.tile_pool(name="warm_ps", bufs=1, space="PSUM") as warm_ps:
                wt = warm_ps.tile([P, 512], F32, tag="warm")
                for _ in range(7):
                    nc.tensor.matmul(
                        wt, lhsT=dummy[:, :P], rhs=dummy, start=True, stop=True
                    )

            with (
                tc.tile_pool(name="proj_ps", bufs=2, space="PSUM") as proj_ps,
                tc.tile_pool(name="st_ps", bufs=2, space="PSUM") as st_ps,
            ):
                # ---------- emission helpers ----------
                def q_chunk(base, width):
                    eb, off = (evoA_sb, 0) if base < 1024 else (evoB_sb, 1024)
                    pt = proj_ps.tile([P, 1024], F32, tag="b")
                    for c in (0, 2):
                        nc.tensor.matmul(
                            pt[:QK, :width],
                            lhsT=evoA_sb[:, c : c + 2, WQO : WQO + QK],
                            rhs=eb[:, c : c + 2, 2 + base - off : 2 + base - off + width],
                            start=(c == 0),
                            stop=(c == 2),
                            perf_mode=DR,
                        )
                    nc.vector.tensor_scalar(
                        out=qt_sb[:, base : base + width],
                        in0=pt[:QK, :width],
                        scalar1=b_sb[:, 0:1],
                        scalar2=None,
                        op0=add,
                    )

                def k_chunk(base, width):
                    eb, off = (evoA_sb, 0) if base < 1024 else (evoB_sb, 1024)
                    pt = proj_ps.tile([P, 1024], F32, tag="b")
                    n = 0
                    for t in range(5):
                        for c in (0, 2):
                            nc.tensor.matmul(
                                pt[:QK, :width],
                                lhsT=evoA_sb[
                                    :, c : c + 2, WKO + QK * t : WKO + QK * (t + 1)
                                ],
                                rhs=eb[:, c : c + 2, t + base - off : t + base - off + width],
                                start=(n == 0),
                                stop=(n == 9),
                                perf_mode=DR,
                            )
                            n += 1
                    nc.vector.tensor_scalar(
                        out=kt_sb[:, base : base + width],
                        in0=pt[:QK, :width],
                        scalar1=b_sb[:, 1:2],
                        scalar2=None,
                        op0=add,
                    )

                def s_tile(j, h):
                    hb = h * HALF
                    stp = st_ps.tile([P, HALF], F32, tag="stp")
                    for o2, w2 in _chunks(HALF, 512):
                        nc.tensor.matmul(
                            stp[:, o2 : o2 + w2],
                            lhsT=kt_sb[:, j * P : (j + 1) * P],
                            rhs=qt_sb[:, hb + o2 : hb + o2 + w2],
                            start=True,
                            stop=True,
                        )
                    nc.scalar.activation(
                        out=et_sb[:, j, hb : hb + HALF],
                        in_=stp,
                        func=mybir.ActivationFunctionType.Exp,
                        bias=mask_sb[:, j : j + 1],
                        scale=NORM,
                    )

                def v_chunk(base, width):
                    pt = proj_ps.tile([P, 1024], F32, tag="b")
                    for c in range(8):
                        nc.tensor.matmul(
                            pt[:QK, :width],
                            lhsT=plm_sb[:, c, 2048 : 2048 + QK],
                            rhs=plm_sb[:, c, base : base + width],
                            start=(c == 0),
                            stop=(c == 7),
                        )
                    nc.vector.tensor_scalar(
                        out=vt_sb[:, base : base + width],
                        in0=pt[:QK, :width],
                        scalar1=b_sb[:, 2:3],
                        scalar2=None,
                        op0=add,
                    )

                def v1_tile(j):
                    # PE transpose into a shared-pool slot viewed as fp16
                    ptf = proj_ps.tile([P, 1024], F32, tag="b")
                    vp = ptf.bitcast(F16)[:, :QK]
                    nc.tensor.transpose(
                        vp,
                        vt_sb[:, j * P : (j + 1) * P],
                        plm_sb[:QK, 1, 2144 : 2144 + QK],
                    )
                    nc.vector.tensor_copy(out=v1_sb[:, j, :QK], in_=vp)

                pairs = [(j, min(2, nkt - j)) for j in range(0, nkt, 2)]

                def o_half(h):
                    hb = h * HALF
                    otp = proj_ps.tile([P, 1024], F32, tag="b")
                    for o2, w2 in _chunks(HALF, 512):
                        for pi, (j, m) in enumerate(pairs):
                            if m == 2:
                                nc.tensor.matmul(
                                    otp[:, o2 : o2 + w2],
                                    lhsT=v1_sb[:, j : j + 2, :],
                                    rhs=et_sb[:, j : j + 2, hb + o2 : hb + o2 + w2],
                                    start=(pi == 0),
                                    stop=(pi == len(pairs) - 1),
                                    perf_mode=DR,
                                )
                            else:
                                nc.tensor.matmul(
                                    otp[:, o2 : o2 + w2],
                                    lhsT=v1_sb[:, j, :],
                                    rhs=et_sb[:, j, hb + o2 : hb + o2 + w2],
                                    start=(pi == 0),
                                    stop=(pi == len(pairs) - 1),
                                )
                    # split drain across DVE + Act (both near-idle at the tail)
                    nc.vector.tensor_copy(
                        out=ot_sb[:, hb : hb + 512], in_=otp[: QK + 1, :512]
                    )
                    nc.scalar.copy(
                        out=ot_sb[:, hb + 512 : hb + HALF],
                        in_=otp[: QK + 1, 512:HALF],
                    )

                # ---------- emission ----------
                # S(j, h0) needs only QT cols 0..1023 (chunks 0/1, from evoA)
                q_chunk(0, 512)
                q_chunk(512, 512)

                # K chunks interleaved with S(h0) for newly covered key tiles;
                # Q chunks 2/3 (evoB) slot in after the first S batch
                done_j = 0
                kch = _chunks(lkw, 512)
                for ci, (base, width) in enumerate(kch):
                    k_chunk(base, width)
                    avail = (base + width) // P
                    for j in range(done_j, avail):
                        s_tile(j, 0)
                    done_j = avail
                    if ci == 1:
                        q_chunk(1024, 512)
                        q_chunk(1536, 512)

                # S(h1) with V proj / V1 transposes / O(h0) interleaved into
                # the exp-backpressure stalls
                jobs2 = (
                    [("v", c) for c in _chunks(L, 512)]
                    + [("x", j) for j in range(nkt)]
                    + [("vtout", 0), ("vtout", 1)]
                    + [("o0", None)]
                )
                sh1 = list(range(nkt))
                ji = 0
                # lead with a couple of S(h1) tiles so the Act queue never dries
                for j in sh1[:2]:
                    s_tile(j, 1)
                for idx, j in enumerate(sh1[2:], start=2):
                    s_tile(j, 1)
                    # ~1 background job per S tile keeps PE busy during waits
                    take = 2 if idx >= len(sh1) - 3 else 1
                    for _ in range(take):
                        if ji < len(jobs2):
                            kind, arg = jobs2[ji]
                            ji += 1
                            if kind == "v":
                                v_chunk(*arg)
                            elif kind == "x":
                                v1_tile(arg)
                            elif kind == "vtout":
                                h = arg
                                nc.gpsimd.dma_start(
                                    out=vt_out[:, h * HALF : (h + 1) * HALF],
                                    in_=vt_sb[:, h * HALF : (h + 1) * HALF],
                                )
                            else:
                                o_half(0)
                while ji < len(jobs2):
                    kind, arg = jobs2[ji]
                    ji += 1
                    if kind == "v":
                        v_chunk(*arg)
                    elif kind == "x":
                        v1_tile(arg)
                    elif kind == "vtout":
                        h = arg
                        nc.gpsimd.dma_start(
                            out=vt_out[:, h * HALF : (h + 1) * HALF],
                            in_=vt_sb[:, h * HALF : (h + 1) * HALF],
                        )
                    else:
                        o_half(0)
                # O(h0) outputs
                nc.gpsimd.dma_start(out=ot_out[:48, :HALF], in_=ot_sb[:48, :HALF])
                nc.sync.dma_start(
                    out=ot_out[48 : QK + 1, :HALF], in_=ot_sb[48 : QK + 1, :HALF]
                )
                # O(h1): final output split across all three queues
                o_half(1)
                nc.gpsimd.dma_start(out=ot_out[:32, HALF:], in_=ot_sb[:32, HALF:])
                nc.sync.dma_start(out=ot_out[32:64, HALF:], in_=ot_sb[32:64, HALF:])
                nc.scalar.dma_start(
                    out=ot_out[64 : QK + 1, HALF:], in_=ot_sb[64 : QK + 1, HALF:]
                )
    nc.finalize()
    return nc


def _prep_core_inputs(evo, plm, seqlen, weights, nkt):
    evoT = np.zeros((Q_IN, L + 4), np.float32)
    evoT[:, 2 : 2 + L] = evo.T
    evo8 = evoT.reshape(4, P, L + 4).transpose(1, 0, 2).astype(NPF8)
    j = np.arange(nkt)[None, :]
    p = np.arange(P)[:, None]
    mask = np.where(j * P + p < seqlen, 0.0, -1e6).astype(np.float32)
    evoA = np.zeros((P, 4, EW), NPF8)
    evoA[:, :, :EC] = evo8[:, :, :EC]
    evoA[:, :, WQO : WQO + QK] = weights["wq"]
    evoA[:, :, WKO : WKO + 5 * QK] = weights["wk"]
    evoA[:QK, 0, BQO : BQO + 12].view(np.float32)[:] = weights["bqkv"]
    evoA[:, 0, MKO : MKO + 4 * nkt].view(np.float32)[:] = mask
    evoB = np.ascontiguousarray(evo8[:, :, L + 4 - EC :])
    plm16 = np.zeros((P, 8, PW), np.float16)
    plm16[:, :, :L] = plm.T.reshape(8, P, L).transpose(1, 0, 2)
    plm16[:, :, 2048 : 2048 + QK] = weights["wv"]
    plm16[:QK, 1, 2144 : 2144 + QK] = np.eye(QK, dtype=np.float16)
    return {"evoA": evoA, "evoB": evoB, "plm": plm16}


def _pack_w(w, n, dt):
    # (n*128, 96) f32 -> (128, n, 96) in the SBUF [p, n, o] layout
    return np.ascontiguousarray(w.reshape(n, P, QK).transpose(1, 0, 2)).astype(dt)


def kernel(
    plm_embedding,
    evo_local,
    seqlengths,
    q_w,
    q_b,
    k_w,
    k_b,
    v_w,
    v_b,
    cn3_w,
    cn3_b,
    cn5_w,
    cn5_b,
):
    global LAST_EXEC_TIME_NS, LAST_RESULTS
    plm_embedding = np.asarray(plm_embedding, np.float32)
    evo_local = np.asarray(evo_local, np.float32)
    seqlengths = np.asarray(seqlengths)

    taps, bk = _fold_k_weights(
        np.asarray(k_w, np.float32),
        np.asarray(k_b, np.float32),
        np.asarray(cn3_w, np.float32),
        np.asarray(cn3_b, np.float32),
        np.asarray(cn5_w, np.float32),
        np.asarray(cn5_b, np.float32),
    )
    nkt = int(min(L // P, (int(seqlengths.max()) + P - 1) // P))
    bqkv = np.stack(
        [np.asarray(q_b, np.float32), bk, np.asarray(v_b, np.float32)], axis=1
    ).astype(np.float32)
    wk20 = _pack_w(taps.reshape(5 * Q_IN, QK), 20, NPF8)  # [P, 20, 96] t-major
    weights = {
        "wq": _pack_w(np.ascontiguousarray(np.asarray(q_w, np.float32).T), 4, NPF8),
        # blob wants [P, c, t, 96]: c-major pairs with taps adjacent per c
        "wk": np.ascontiguousarray(
            wk20.reshape(P, 5, 4, QK).transpose(0, 2, 1, 3).reshape(P, 4, 5 * QK)
        ),
        "wv": _pack_w(
            np.ascontiguousarray(np.asarray(v_w, np.float32).T), 8, np.float16
        ),
        "bqkv": np.ascontiguousarray(bqkv),
    }

    if nkt not in _program_cache:
        _program_cache[nkt] = _build_program(nkt)
    nc = _program_cache[nkt]

    in_maps = [
        _prep_core_inputs(
            evo_local[b], plm_embedding[b], int(seqlengths[b]), weights, nkt
        )
        for b in range(B)
    ]
    trace = bool(os.environ.get("KBENCH_TRACE"))
    res = run_bass_kernel_spmd(nc, in_maps, list(range(B)), trace=trace)
    LAST_EXEC_TIME_NS = res.exec_time_ns
    LAST_RESULTS = res

    out = np.empty((B, L, VD), np.float32)
    for b in range(B):
        ot = res.results[b]["ot"].astype(np.float32)
        vt = res.results[b]["vt"].astype(np.float32)
        out[b] = (ot[:QK] / ot[QK : QK + 1]).T + vt.T
    return out
